# revision 17
# baseline (speedup 1.0000x reference)
"""ConvLSTM (B=4, T=16, C=1, H=W=64, HID=64, 2 layers) on 8 Trainium2 NeuronCores.

Sharding: batch (4) x H-halves (2) -> 8 cores. Each core owns a 32-row half
of one batch element. The recurrence is sequential in T; per step the two
H-half cores exchange a 1-row halo of h0 / h1 via tiny 2-rank AllGathers
(replica groups [[0,1],[2,3],[4,5],[6,7]]) which hide behind compute.

Odd (bottom-half) cores run on vertically flipped data (host flips x, the
conv kernels over ky, and the output back) so the device program is fully
symmetric across cores (same plane row sends/receives the halo everywhere).

Convs run on the PE as fp32r matmuls (1 cycle/row at N=512):
 - layer1: 9 shifts x K=128 matmuls, rhs = [h1 ; h0'] stacked plane tile.
 - layer0: 5 K=128 matmul slots: 3x shift-pairs sharing a delta=+1 doubled
   plane, 1x pair sharing delta=+66, and 1x {shift, x-taps} pair (the C=1
   x contribution folds into K-rows 64..72 via host-built im2col planes).
Gate math: ACT sigmoid/tanh (conv bias folded into the activation bias);
the o-gate of layer0 is pre-scaled 0.5 so one tanh covers [g, o/2] and
h0' = 2*h0 = (tanh(o/2)+1)*tanh(c) in one fused scalar_tensor_tensor op
(the 2x is folded into consumer weights). The one cross-partition-half add
of the c update (i*tanh(g) into f*c) is a SWDGE accumulate-DMA.
"""

import os
import numpy as np

import concourse.bass as bass
import concourse.mybir as mybir
import concourse.tile as tile
from concourse import bacc
from concourse import bass_utils
from concourse.alu_op_type import AluOpType

# Problem constants (hardcoded per contract)
B, T_FULL, C, H, W = 4, 16, 1, 64, 64
HID = 64
N_CORES = 8
HH = 32            # rows per H-half
PR, PC = 34, 66    # padded plane rows/cols (32+2, 64+2)
PL = PR * PC       # 2244 floats per partition per plane
SP = HH * W        # 2048 spatial positions per core
BANDS = 2          # 16-row bands
F32 = mybir.dt.float32
F32R = mybir.dt.float32r
ACT_SIG = mybir.ActivationFunctionType.Sigmoid
ACT_TANH = mybir.ActivationFunctionType.Tanh

# Layer-0 matmul slots: (shift_a, shift_b) pairs sharing a doubled plane, or
# (shift_a, 'X') pairing the leftover shift with the x-tap rows.
SLOTS0 = [
    ((-1, -1), (-1, 0), "DBL1"),   # delta +1
    ((0, -1), (0, 0), "DBL1"),
    ((1, -1), (1, 0), "DBL1"),
    ((-1, 1), (0, 1), "DBL3"),     # delta +66
    ((1, 1), "X", "DBL2"),
]
SHIFTS = [(dy, dx) for dy in (-1, 0, 1) for dx in (-1, 0, 1)]
REPLICA_GROUPS = [[0, 1], [2, 3], [4, 5], [6, 7]]


# ---------------------------------------------------------------- host prep

def _pack_weights_l0(w0, b0):
    """Build [5, 2, 128, 128] lhsT slots + [128, 2] bias for layer 0.

    Block0 co rows = [i, f]; block1 = [g, 0.5*o]. h-part K rows carry an
    extra 0.5 (h0' = 2*h0 compensation).
    """
    co_map = np.concatenate([np.arange(0, 64), np.arange(64, 128),
                             np.arange(192, 256), np.arange(128, 192)])
    scale = np.ones(256, np.float32)
    scale[192:256] = 0.5  # o-gate rows (positions 192.. in reordered layout)
    w0r = w0[co_map] * scale[:, None, None, None]   # [256, 65, 3, 3]
    w0h = w0r[:, 1:65] * 0.5                        # h-part, [256, 64, 3, 3]
    w0x = w0r[:, 0]                                 # x-part, [256, 3, 3]
    b0r = b0[co_map] * scale
    w0s = np.zeros((5, 2, 128, 128), np.float32)
    for sl, (sa, sb, _) in enumerate(SLOTS0):
        for cb in range(2):
            rows = slice(cb * 128, (cb + 1) * 128)
            w0s[sl, cb, 0:64, :] = w0h[rows, :, sa[0] + 1, sa[1] + 1].T
            if sb == "X":
                for ky in range(3):
                    for kx in range(3):
                        w0s[sl, cb, 64 + ky * 3 + kx, :] = w0x[rows, ky, kx]
            else:
                w0s[sl, cb, 64:128, :] = w0h[rows, :, sb[0] + 1, sb[1] + 1].T
    b0p = b0r.reshape(2, 128).T.copy()  # [128, 2]
    return w0s, b0p


def _pack_weights_l1(w1, b1):
    """[9, 2, 128, 128] lhsT per shift + [128, 2] bias for layer 1.

    Block0 co rows = [f, i]; block1 = [0.5*o, g] so one tanh covers the
    block (tanh(o/2) = 2*sig(o)-1); the device stores h1' = 2*h1 =
    (tanh(o/2)+1)*tanh(c1) and the host halves y. K rows =
    [0.5*h1' ; 0.5*h0']. Reference combined = [h0 ; h1]."""
    co_map = np.concatenate([np.arange(64, 128), np.arange(0, 64),
                             np.arange(128, 192), np.arange(192, 256)])
    scale = np.ones(256, np.float32)
    scale[128:192] = 0.5
    w1r = w1[co_map] * scale[:, None, None, None]   # [256, 128, 3, 3]
    w1h1 = w1r[:, 64:128] * 0.5
    w1h0 = w1r[:, 0:64] * 0.5
    b1p = (b1[co_map] * scale).reshape(2, 128).T.copy()
    w1s = np.zeros((9, 2, 128, 128), np.float32)
    for si, (dy, dx) in enumerate(SHIFTS):
        ky, kx = dy + 1, dx + 1
        for cb in range(2):
            rows = slice(cb * 128, (cb + 1) * 128)
            w1s[si, cb, 0:64, :] = w1h1[rows, :, ky, kx].T
            w1s[si, cb, 64:128, :] = w1h0[rows, :, ky, kx].T
    return w1s, b1p


def _build_xim(x_local34, t_steps):
    """x im2col planes [T, 9, PL] aligned to the slot-4 read offset (1,+1).

    x_local34: [T, 34, 64] plane rows (row 0 = edge zero, 1..32 = own rows,
    33 = x halo row from the neighboring half).
    xim[k][r', c'] = xpad[r' + ky - 2, c' + kx - 2]  (k = ky*3+kx)
    """
    xim = np.zeros((t_steps, 9, PR, PC), np.float32)
    xpad = np.zeros((t_steps, PR, PC), np.float32)
    xpad[:, :, 1:65] = x_local34
    for ky in range(3):
        for kx in range(3):
            k = ky * 3 + kx
            dr, dc = ky - 2, kx - 2
            r0, r1 = max(0, -dr), min(PR, PR - dr)
            c0, c1 = max(0, -dc), min(PC, PC - dc)
            xim[:, k, r0:r1, c0:c1] = xpad[:, r0 + dr:r1 + dr, c0 + dc:c1 + dc]
    return xim.reshape(t_steps, 9, PL)


def prep_core_inputs(x, w0, b0, w1, b1, core, t_steps):
    b, half = core // 2, core % 2
    xb = np.asarray(x[b, :t_steps, 0], np.float32)      # [T, 64, 64]
    x_local = np.zeros((t_steps, PR, W), np.float32)
    if half == 0:
        x_local[:, 1:34, :] = xb[:, 0:HH + 1, :]        # own rows + halo row 32
        w0f, w1f = w0, w1
    else:
        x_local[:, 1:34, :] = xb[:, HH - 1:2 * HH, :][:, ::-1, :]
        w0f, w1f = w0[:, :, ::-1, :], w1[:, :, ::-1, :]
    w0s, b0p = _pack_weights_l0(np.asarray(w0f, np.float32), np.asarray(b0, np.float32))
    w1s, b1p = _pack_weights_l1(np.asarray(w1f, np.float32), np.asarray(b1, np.float32))
    xim = _build_xim(x_local, t_steps)
    return {
        "w0s": np.ascontiguousarray(w0s),
        "w1s": np.ascontiguousarray(w1s),
        "b0p": np.ascontiguousarray(b0p),
        "b1p": np.ascontiguousarray(b1p),
        "xim": np.ascontiguousarray(xim),
        "zz": np.zeros((128, PL), np.float32),
    }


def assemble_output(results, t_steps):
    y = np.zeros((B, t_steps, HID, H, W), np.float32)
    for core in range(N_CORES):
        b, half = core // 2, core % 2
        ys = results[core]["y"].reshape(t_steps, HID, HH, W) * 0.5
        if half == 0:
            y[b, :, :, 0:HH, :] = ys
        else:
            y[b, :, :, HH:2 * HH, :] = ys[:, :, ::-1, :]
    return y


# ------------------------------------------------------------- device build

def _pview(t, p0, p1):
    """[p, PR, PC] view of a plane tile slice."""
    return t[p0:p1, :].rearrange("p (r c) -> p r c", r=PR, c=PC)


def _band(t, p0, p1, band):
    """[64, 16, 64] view of band `band` of a [128, 2048] tile slice."""
    return t[p0:p1, band * 1024:(band + 1) * 1024].rearrange(
        "p (r c) -> p r c", r=16, c=64)


def build_nc(t_steps):
    nc = bacc.Bacc("TRN2", target_bir_lowering=False, debug=False,
                   num_devices=N_CORES)
    d_w0s = nc.dram_tensor("w0s", [5, 2, 128, 128], F32R, kind="ExternalInput")
    d_w1s = nc.dram_tensor("w1s", [9, 2, 128, 128], F32R, kind="ExternalInput")
    d_b0 = nc.dram_tensor("b0p", [128, 2], F32, kind="ExternalInput")
    d_b1 = nc.dram_tensor("b1p", [128, 2], F32, kind="ExternalInput")
    d_xim = nc.dram_tensor("xim", [t_steps, 9, PL], F32R, kind="ExternalInput")
    d_zz = nc.dram_tensor("zz", [128, PL], F32R, kind="ExternalInput")
    d_y = nc.dram_tensor("y", [t_steps, HID, SP], F32R, kind="ExternalOutput")

    with tile.TileContext(nc) as tc:
        with (
            tc.tile_pool(name="persist", bufs=1) as pp,
            tc.tile_pool(name="gates", bufs=2) as gp,
            tc.tile_pool(name="psum", bufs=8, space="PSUM") as psp,
            tc.tile_pool(name="dram", bufs=2, space="DRAM") as dp,
        ):
            # persistent planes (f32r; matmul inputs)
            dbl1 = pp.tile([128, PL], F32R, tag="dbl1")
            dbl2 = pp.tile([128, PL], F32R, tag="dbl2")
            dbl3 = pp.tile([128, PL], F32R, tag="dbl3")
            l1in = pp.tile([128, PL], F32R, tag="l1in")
            # dbl1/dbl3 need no zero-init: t=0 uses only slot 4 (dbl2), and
            # the gates0(0) refresh fully rewrites them before conv0(1)
            for pl_t in (dbl2, l1in):
                nc.sync.dma_start(pl_t[:], d_zz.ap())
            # weights; slot 4 (the only t=0 slot) loads first
            w0t = pp.tile([128, 5 * 2 * 128], F32R, tag="w0t")
            nc.sync.dma_start(
                w0t[:, 8 * 128:10 * 128].rearrange(
                    "p (c m) -> p c m", c=2),
                d_w0s.ap()[4].rearrange("c k m -> k c m"))
            nc.sync.dma_start(
                w0t[:, 0:8 * 128].rearrange("p (s c m) -> p s c m", s=4, c=2),
                d_w0s.ap()[0:4].rearrange("s c k m -> k s c m"))
            w1t = pp.tile([128, 9 * 2 * 128], F32R, tag="w1t")
            nc.sync.dma_start(
                w1t[:].rearrange("p (s c m) -> p s c m", s=9, c=2),
                d_w1s.ap().rearrange("s c k m -> k s c m"))
            b0t = pp.tile([128, 2], F32, tag="b0t")
            nc.sync.dma_start(b0t[:], d_b0.ap())
            b1t = pp.tile([128, 2], F32, tag="b1t")
            nc.sync.dma_start(b1t[:], d_b1.ap())
            # first x im2col
            nc.sync.dma_start(dbl2[64:73, :], d_xim.ap()[0])

            def w0ap(sl, cb):
                return w0t[:, (sl * 2 + cb) * 128:(sl * 2 + cb + 1) * 128]

            def w1ap(si, cb):
                return w1t[:, (si * 2 + cb) * 128:(si * 2 + cb + 1) * 128]

            dbl_map = {"DBL1": dbl1, "DBL2": dbl2, "DBL3": dbl3}

            c_prev = gp.tile([128, SP], F32, tag="cpair")
            nc.any.memset(c_prev[:], 0.0)

            # --------------------------------------------------------------
            # Software-pipelined schedule. PE stream per iteration t:
            #   ... conv1(t) [bands 1,0], conv0(t+1) [bands 1,0] ...
            # gates1(t) (scalar/vector) overlaps conv0(t+1); gates0(t+1)
            # overlaps conv0(t+1)'s band-0 tail; the h0/h1 halo AllGathers
            # are kicked from a row-32-only mini gate chain right after the
            # producing band's psums land, hiding CC latency under matmuls.
            # PSUM tiles are [128,512] (1 bank) so conv1(t)/conv0(t+1)
            # lifetimes interleave within the 8 banks.
            # --------------------------------------------------------------

            def conv0(t):
                """Layer-0 conv for step t -> {(band, cb, sl): psum}."""
                ps0 = {}
                for band in (1, 0):
                    for cb in range(2):
                        for sl in range(2):
                            ps = psp.tile([128, 512], F32)
                            ps0[(band, cb, sl)] = ps
                            r0 = band * 16 + sl * 8 + 1
                            slots = range(5) if t > 0 else [4]
                            first = 0 if t > 0 else 4
                            for slot in slots:
                                sa, sb_, tname = SLOTS0[slot]
                                src = dbl_map[tname]
                                rhs = _pview(src, 0, 128)[
                                    :, r0 + sa[0]:r0 + sa[0] + 8,
                                    1 + sa[1]:1 + sa[1] + 64]
                                nc.tensor.matmul(
                                    ps[:], w0ap(slot, cb), rhs,
                                    start=(slot == first), stop=(slot == 4))
                return ps0

            def conv1(t):
                """Layer-1 conv for step t -> {(band, cb, sl): psum}.

                Shift order defers cross-band / halo row reads: band-1 sl0
                does dy=-1 (reads row 16 <- gates0 band 0) last; band-1 sl1
                does dy=+1 (reads halo row 33 <- AllGather) last.
                """
                ps1 = {}
                for band in (1, 0):
                    for cb in range(2):
                        for sl in range(2):
                            ps = psp.tile([128, 512], F32)
                            ps1[(band, cb, sl)] = ps
                            r0 = band * 16 + sl * 8 + 1
                            if band == 1 and sl == 0:
                                order = [s for s in SHIFTS if s[0] != -1] + \
                                        [s for s in SHIFTS if s[0] == -1]
                            elif band == 1 and sl == 1:
                                order = [s for s in SHIFTS if s[0] != 1] + \
                                        [s for s in SHIFTS if s[0] == 1]
                            else:
                                order = SHIFTS
                            for si, (dy, dx) in enumerate(order):
                                rhs = _pview(l1in, 0, 128)[
                                    :, r0 + dy:r0 + dy + 8, 1 + dx:1 + dx + 64]
                                nc.tensor.matmul(
                                    ps[:], w1ap(SHIFTS.index((dy, dx)), cb),
                                    rhs, start=(si == 0), stop=(si == 8))
                return ps1

            def halo_send(half, tag):
                """AllGather plane row 32 of l1in half -> returns recv state."""
                p0, p1 = (64, 128) if half else (0, 64)
                ag_in = dp.tile([64, 64], F32R, tag=f"ag{tag}i")
                ag_out = dp.tile([128, 64], F32R, tag=f"ag{tag}o")
                nc.sync.dma_start(ag_in[:], _pview(l1in, p0, p1)[:, 32, 1:65])
                nc.gpsimd.collective_compute(
                    "AllGather", AluOpType.bypass,
                    replica_groups=REPLICA_GROUPS,
                    ins=[ag_in.opt()], outs=[ag_out.opt()])
                return ag_out

            def halo_recv(half, tag, ag_out):
                """Write peer's row into l1in plane row 33 (sum-minus-own)."""
                p0, p1 = (64, 128) if half else (0, 64)
                agt = gp.tile([128, 128], F32R, tag=f"agt{tag}")
                nc.sync.dma_start(
                    agt[p0:p1, :].rearrange("p (s c) -> p s c", s=2),
                    ag_out[:].rearrange("(s p) c -> p s c", s=2))
                st = gp.tile([128, 64], F32R, tag=f"st{tag}")
                nc.vector.tensor_add(st[p0:p1, :], agt[p0:p1, 0:64],
                                     agt[p0:p1, 64:128])
                nc.vector.tensor_sub(
                    _pview(l1in, p0, p1)[:, 33, 1:65], st[p0:p1, :],
                    _pview(l1in, p0, p1)[:, 32, 1:65])

            def chain0a(ps0, c_new, sig0, tgo0, t2p, band, sl, c0, c1):
                """Layer-0 gates phase A: activations, c-muls, accum-DMA.

                SP cols = band*1024+sl*512+c0. The i*tanh(g) mul runs on
                gpsimd right before its accum on the same queue."""
                s0 = band * 1024 + sl * 512 + c0
                bs = slice(s0, s0 + (c1 - c0))
                nc.scalar.activation(sig0[:, bs], ps0[(band, 0, sl)][:, c0:c1],
                                     ACT_SIG, bias=b0t[:, 0:1])
                nc.scalar.activation(tgo0[:, bs], ps0[(band, 1, sl)][:, c0:c1],
                                     ACT_TANH, bias=b0t[:, 1:2])
                nc.vector.tensor_mul(c_new[64:128, bs], sig0[64:128, bs],
                                     c_prev_ref[0][64:128, bs])
                nc.gpsimd.tensor_mul(t2p[0:64, bs], sig0[0:64, bs],
                                     tgo0[0:64, bs])
                nc.gpsimd.dma_start(c_new[64:128, bs], t2p[0:64, bs],
                                    accum_op=AluOpType.add)

            def chain0b(c_new, tgo0, thc, band, sl, c0, c1):
                """Layer-0 gates phase B: tanh(c0) and h0' -> l1in hi."""
                s0 = band * 1024 + sl * 512 + c0
                bs = slice(s0, s0 + (c1 - c0))
                nc.scalar.activation(thc[64:128, bs], c_new[64:128, bs],
                                     ACT_TANH)
                r0 = band * 16 + sl * 8 + 1 + c0 // 64
                rows = _pview(l1in, 64, 128)[:, r0:r0 + (c1 - c0) // 64, 1:65]
                nc.vector.scalar_tensor_tensor(
                    rows,
                    tgo0[64:128, bs].rearrange("p (r c) -> p r c", c=64), 1.0,
                    thc[64:128, bs].rearrange("p (r c) -> p r c", c=64),
                    AluOpType.add, AluOpType.mult)

            def chain1a(ps1, c_new, sig1, og1, t2p, band, sl, c0, c1):
                """Layer-1 gates phase A."""
                s0 = band * 1024 + sl * 512 + c0
                bs = slice(s0, s0 + (c1 - c0))
                nc.scalar.activation(sig1[:, bs], ps1[(band, 0, sl)][:, c0:c1],
                                     ACT_SIG, bias=b1t[:, 0:1])
                nc.scalar.activation(og1[:, bs], ps1[(band, 1, sl)][:, c0:c1],
                                     ACT_TANH, bias=b1t[:, 1:2])
                nc.vector.tensor_mul(c_new[0:64, bs], sig1[0:64, bs],
                                     c_prev_ref[0][0:64, bs])
                nc.gpsimd.tensor_mul(t2p[64:128, bs], sig1[64:128, bs],
                                     og1[64:128, bs])
                nc.gpsimd.dma_start(c_new[0:64, bs], t2p[64:128, bs],
                                    accum_op=AluOpType.add)

            def chain1b(c_new, og1, thc, band, sl, c0, c1):
                """Layer-1 gates phase B: tanh(c1), h1' = 2*h1 -> l1in lo."""
                s0 = band * 1024 + sl * 512 + c0
                bs = slice(s0, s0 + (c1 - c0))
                nc.scalar.activation(thc[0:64, bs], c_new[0:64, bs],
                                     ACT_TANH)
                r0 = band * 16 + sl * 8 + 1 + c0 // 64
                rows = _pview(l1in, 0, 64)[:, r0:r0 + (c1 - c0) // 64, 1:65]
                # h1' = 2*h1 = (tanh(o/2)+1)*tanh(c1); host halves y
                nc.vector.scalar_tensor_tensor(
                    rows,
                    og1[0:64, bs].rearrange("p (r c) -> p r c", c=64), 1.0,
                    thc[0:64, bs].rearrange("p (r c) -> p r c", c=64),
                    AluOpType.add, AluOpType.mult)

            c_prev_ref = [c_prev]

            # prologue: layer-0 conv for t=0 (x taps only; h is zero)
            ps0 = conv0(0)

            for t in range(t_steps):
                # ---- gates0(t): band 1 (mini row 32 first -> AG), band 0
                sig0 = gp.tile([128, SP], F32, tag="sig0")
                tgo0 = gp.tile([128, SP], F32, tag="tgo0")
                c_new = gp.tile([128, SP], F32, tag="cpair")
                t2p = gp.tile([128, SP], F32, tag="t2p")
                thc = gp.tile([128, SP], F32, tag="thc")

                # mini row-32 chain first -> early AllGather kick
                chain0a(ps0, c_new, sig0, tgo0, t2p, 1, 1, 448, 512)
                chain0b(c_new, tgo0, thc, 1, 1, 448, 512)
                ag0 = halo_send(1, "0")
                # A/B staggered so the scalar queue never waits on accum-DMAs;
                # (1,1)/(1,0) complete first (conv1 band 1 needs rows 17-32)
                chain0a(ps0, c_new, sig0, tgo0, t2p, 1, 1, 0, 448)
                chain0a(ps0, c_new, sig0, tgo0, t2p, 1, 0, 0, 512)
                chain0b(c_new, tgo0, thc, 1, 1, 0, 448)
                chain0b(c_new, tgo0, thc, 1, 0, 0, 512)
                # band 0: sl1 first (conv1 band 1 needs plane row 16)
                chain0a(ps0, c_new, sig0, tgo0, t2p, 0, 1, 0, 512)
                chain0a(ps0, c_new, sig0, tgo0, t2p, 0, 0, 0, 512)
                chain0b(c_new, tgo0, thc, 0, 1, 0, 512)
                chain0b(c_new, tgo0, thc, 0, 0, 0, 512)

                # refresh layer-0 rhs planes for step t+1 (feeds conv0(t+1)).
                # Big copies stop before plane row 33 (flat col 33*66=2178)
                # so they don't wait on the halo; tiny row-33 tails go after
                # halo_recv below.
                R33 = 33 * 66
                if t + 1 < t_steps:
                    nc.sync.dma_start(dbl1[0:64, 0:R33], l1in[64:128, 0:R33])
                    nc.sync.dma_start(dbl1[64:128, 0:R33],
                                      l1in[64:128, 1:R33 + 1])
                    nc.sync.dma_start(dbl3[0:64, 0:R33], l1in[64:128, 0:R33])
                    nc.sync.dma_start(dbl3[64:128, 0:R33 - 66],
                                      l1in[64:128, 66:R33])
                    nc.sync.dma_start(dbl2[0:64, 0:R33], l1in[64:128, 0:R33])
                    nc.sync.dma_start(dbl2[64:73, :], d_xim.ap()[t + 1])

                # recv late so the vector queue never stalls on the AG
                halo_recv(1, "0", ag0)
                # row-33 tails of the dbl refresh (need the halo row)
                if t + 1 < t_steps:
                    nc.sync.dma_start(dbl1[0:64, R33:PL], l1in[64:128, R33:PL])
                    nc.sync.dma_start(dbl1[64:128, R33:PL - 1],
                                      l1in[64:128, R33 + 1:PL])
                    nc.sync.dma_start(dbl3[0:64, R33:PL], l1in[64:128, R33:PL])
                    nc.sync.dma_start(dbl3[64:128, R33 - 66:R33],
                                      l1in[64:128, R33:PL])
                    nc.sync.dma_start(dbl2[0:64, R33:PL], l1in[64:128, R33:PL])

                # ---- conv1(t) on PE (after conv0(t) in the queue)
                ps1 = conv1(t)

                # ---- gates1(t): overlaps conv0(t+1) on PE
                sig1 = gp.tile([128, SP], F32, tag="sig1")
                og1 = gp.tile([128, SP], F32, tag="og1")

                chain1a(ps1, c_new, sig1, og1, t2p, 1, 1, 448, 512)
                chain1b(c_new, og1, thc, 1, 1, 448, 512)
                if t + 1 < t_steps:
                    ag1 = halo_send(0, "1")
                chain1a(ps1, c_new, sig1, og1, t2p, 1, 1, 0, 448)
                chain1a(ps1, c_new, sig1, og1, t2p, 1, 0, 0, 512)
                chain1b(c_new, og1, thc, 1, 1, 0, 448)
                chain1b(c_new, og1, thc, 1, 0, 0, 512)
                # y band 1
                nc.sync.dma_start(
                    d_y.ap()[t][:, 1024:2048].rearrange(
                        "p (r c) -> p r c", r=16, c=64),
                    _pview(l1in, 0, 64)[:, 17:33, 1:65])
                chain1a(ps1, c_new, sig1, og1, t2p, 0, 1, 0, 512)
                chain1a(ps1, c_new, sig1, og1, t2p, 0, 0, 0, 512)
                chain1b(c_new, og1, thc, 0, 1, 0, 512)
                chain1b(c_new, og1, thc, 0, 0, 0, 512)
                if t + 1 < t_steps:
                    halo_recv(0, "1", ag1)
                # y band 0
                nc.sync.dma_start(
                    d_y.ap()[t][:, 0:1024].rearrange(
                        "p (r c) -> p r c", r=16, c=64),
                    _pview(l1in, 0, 64)[:, 1:17, 1:65])

                # ---- conv0(t+1) on PE (independent of gates1(t))
                if t + 1 < t_steps:
                    ps0 = conv0(t + 1)

                c_prev_ref[0] = c_new

    nc.compile()
    return nc


# ------------------------------------------------------------------ driver

def _ensure_axon_ntff_hook():
    """Install the NTFF profile hook bass_utils expects under axon, if the
    environment's antenv lacks it. Only used when tracing is requested."""
    import sys as _sys
    import types as _types
    import ctypes as _ctypes
    import contextlib as _contextlib

    try:
        from antenv.axon_hooks import get_axon_ntff_profile_hook  # noqa: F401
        return
    except ImportError:
        pass
    so_path = "/opt/axon/libaxon_pjrt.so"
    if not os.path.exists(so_path):
        return
    lib = _ctypes.CDLL(so_path)
    if not hasattr(lib, "axon_start_nrt_profile"):
        return
    lib.axon_start_nrt_profile.argtypes = [
        _ctypes.POINTER(_ctypes.c_int64), _ctypes.c_size_t]
    lib.axon_start_nrt_profile.restype = _ctypes.c_int64
    lib.axon_stop_nrt_profile.argtypes = [_ctypes.c_char_p]
    lib.axon_stop_nrt_profile.restype = _ctypes.c_int64

    @_contextlib.contextmanager
    def _hook(output_dir, device_ids):
        import jax
        jax.devices()
        if device_ids:
            ids = (_ctypes.c_int64 * len(device_ids))(*device_ids)
            rc = lib.axon_start_nrt_profile(ids, len(device_ids))
        else:
            rc = lib.axon_start_nrt_profile(None, 0)
        if rc != 0:
            raise RuntimeError(f"axon_start_nrt_profile rc={rc}")
        try:
            yield
        finally:
            n = lib.axon_stop_nrt_profile(str(output_dir).encode())
            print(f"ntff profile: {n} file(s) -> {output_dir}")

    mod = _types.ModuleType("antenv.axon_hooks")
    mod.get_axon_ntff_profile_hook = lambda: _hook
    import antenv  # noqa: F401
    _sys.modules["antenv.axon_hooks"] = mod


_CACHE = {}


def _get_nc(t_steps):
    if t_steps not in _CACHE:
        _CACHE[t_steps] = build_nc(t_steps)
    return _CACHE[t_steps]


def run_cores(x, w0, b0, w1, b1, t_steps=None, trace=False, tmpdir=None):
    t_steps = t_steps or x.shape[1]
    nc = _get_nc(t_steps)
    in_maps = [prep_core_inputs(x, w0, b0, w1, b1, core, t_steps)
               for core in range(N_CORES)]
    kwargs = {}
    if trace:
        _ensure_axon_ntff_hook()
        bass_utils.upload_artifacts = lambda d: d  # no artifact bucket here
        if tmpdir:
            kwargs["tmpdir"] = tmpdir
    res = bass_utils.run_bass_kernel_spmd(
        nc, in_maps, core_ids=list(range(N_CORES)), trace=trace, **kwargs)
    return res


def kernel(x, w0, b0, w1, b1):
    x = np.asarray(x, np.float32)
    t_steps = x.shape[1]
    trace = bool(int(os.environ.get("CONVLSTM_TRACE", "0")))
    res = run_cores(x, np.asarray(w0, np.float32), np.asarray(b0, np.float32),
                    np.asarray(w1, np.float32), np.asarray(b1, np.float32),
                    t_steps=t_steps, trace=trace)
    kernel.last_results = res
    return assemble_output(res.results, t_steps)



# revision 19
# speedup vs baseline: 1.0837x; 1.0837x over previous
"""ConvLSTM (B=4, T=16, C=1, H=W=64, HID=64, 2 layers) on 8 Trainium2 NeuronCores.

Sharding: batch (4) x H-halves (2) -> 8 cores. Each core owns a 32-row half
of one batch element. The recurrence is sequential in T; per step the two
H-half cores exchange a 1-row halo of h0 / h1 via tiny 2-rank AllGathers
(replica groups [[0,1],[2,3],[4,5],[6,7]]) which hide behind compute.

Odd (bottom-half) cores run on vertically flipped data (host flips x, the
conv kernels over ky, and the output back) so the device program is fully
symmetric across cores (same plane row sends/receives the halo everywhere).

Convs run on the PE as fp32r matmuls (1 cycle/row at N=512):
 - layer1: 9 shifts x K=128 matmuls, rhs = [h1 ; h0'] stacked plane tile.
 - layer0: 5 K=128 matmul slots: 3x shift-pairs sharing a delta=+1 doubled
   plane, 1x pair sharing delta=+66, and 1x {shift, x-taps} pair (the C=1
   x contribution folds into K-rows 64..72 via host-built im2col planes).
Gate math: ACT sigmoid/tanh (conv bias folded into the activation bias);
the o-gate of layer0 is pre-scaled 0.5 so one tanh covers [g, o/2] and
h0' = 2*h0 = (tanh(o/2)+1)*tanh(c) in one fused scalar_tensor_tensor op
(the 2x is folded into consumer weights). The one cross-partition-half add
of the c update (i*tanh(g) into f*c) is a SWDGE accumulate-DMA.
"""

import os
import numpy as np

import concourse.bass as bass
import concourse.mybir as mybir
import concourse.tile as tile
from concourse import bacc
from concourse import bass_utils
from concourse.alu_op_type import AluOpType

# Problem constants (hardcoded per contract)
B, T_FULL, C, H, W = 4, 16, 1, 64, 64
HID = 64
N_CORES = 8
HH = 32            # rows per H-half
PR, PC = 34, 66    # padded plane rows/cols (32+2, 64+2)
PL = PR * PC       # 2244 floats per partition per plane
SP = HH * W        # 2048 spatial positions per core
BANDS = 2          # 16-row bands
F32 = mybir.dt.float32
F32R = mybir.dt.float32r
ACT_SIG = mybir.ActivationFunctionType.Sigmoid
ACT_TANH = mybir.ActivationFunctionType.Tanh

# Layer-0 matmul slots: (shift_a, shift_b) pairs sharing a doubled plane, or
# (shift_a, 'X') pairing the leftover shift with the x-tap rows.
SLOTS0 = [
    ((-1, -1), (-1, 0), "DBL1"),   # delta +1
    ((0, -1), (0, 0), "DBL1"),
    ((1, -1), (1, 0), "DBL1"),
    ((-1, 1), (0, 1), "DBL3"),     # delta +66
    ((1, 1), "X", "DBL2"),
]
SHIFTS = [(dy, dx) for dy in (-1, 0, 1) for dx in (-1, 0, 1)]
REPLICA_GROUPS = [[0, 1], [2, 3], [4, 5], [6, 7]]


# ---------------------------------------------------------------- host prep

def _pack_weights_l0(w0, b0):
    """Build [5, 2, 128, 128] lhsT slots + [128, 2] bias for layer 0.

    Block0 co rows = [i, f]; block1 = [g, 0.5*o]. h-part K rows carry an
    extra 0.5 (h0' = 2*h0 compensation).
    """
    co_map = np.concatenate([np.arange(0, 64), np.arange(64, 128),
                             np.arange(192, 256), np.arange(128, 192)])
    scale = np.ones(256, np.float32)
    scale[192:256] = 0.5  # o-gate rows (positions 192.. in reordered layout)
    w0r = w0[co_map] * scale[:, None, None, None]   # [256, 65, 3, 3]
    w0h = w0r[:, 1:65] * 0.5                        # h-part, [256, 64, 3, 3]
    w0x = w0r[:, 0]                                 # x-part, [256, 3, 3]
    b0r = b0[co_map] * scale
    w0s = np.zeros((5, 2, 128, 128), np.float32)
    for sl, (sa, sb, _) in enumerate(SLOTS0):
        for cb in range(2):
            rows = slice(cb * 128, (cb + 1) * 128)
            w0s[sl, cb, 0:64, :] = w0h[rows, :, sa[0] + 1, sa[1] + 1].T
            if sb == "X":
                for ky in range(3):
                    for kx in range(3):
                        w0s[sl, cb, 64 + ky * 3 + kx, :] = w0x[rows, ky, kx]
            else:
                w0s[sl, cb, 64:128, :] = w0h[rows, :, sb[0] + 1, sb[1] + 1].T
    b0p = b0r.reshape(2, 128).T.copy()  # [128, 2]
    return w0s, b0p


def _pack_weights_l1(w1, b1):
    """[9, 2, 128, 128] lhsT per shift + [128, 2] bias for layer 1.

    Block0 co rows = [f, i]; block1 = [0.5*o, g] so one tanh covers the
    block (tanh(o/2) = 2*sig(o)-1); the device stores h1' = 2*h1 =
    (tanh(o/2)+1)*tanh(c1) and the host halves y. K rows =
    [0.5*h1' ; 0.5*h0']. Reference combined = [h0 ; h1]."""
    co_map = np.concatenate([np.arange(64, 128), np.arange(0, 64),
                             np.arange(128, 192), np.arange(192, 256)])
    scale = np.ones(256, np.float32)
    scale[128:192] = 0.5
    w1r = w1[co_map] * scale[:, None, None, None]   # [256, 128, 3, 3]
    w1h1 = w1r[:, 64:128] * 0.5
    w1h0 = w1r[:, 0:64] * 0.5
    b1p = (b1[co_map] * scale).reshape(2, 128).T.copy()
    w1s = np.zeros((9, 2, 128, 128), np.float32)
    for si, (dy, dx) in enumerate(SHIFTS):
        ky, kx = dy + 1, dx + 1
        for cb in range(2):
            rows = slice(cb * 128, (cb + 1) * 128)
            w1s[si, cb, 0:64, :] = w1h1[rows, :, ky, kx].T
            w1s[si, cb, 64:128, :] = w1h0[rows, :, ky, kx].T
    return w1s, b1p


def _build_xim(x_local34, t_steps):
    """x im2col planes [T, 9, PL] aligned to the slot-4 read offset (1,+1).

    x_local34: [T, 34, 64] plane rows (row 0 = edge zero, 1..32 = own rows,
    33 = x halo row from the neighboring half).
    xim[k][r', c'] = xpad[r' + ky - 2, c' + kx - 2]  (k = ky*3+kx)
    """
    xim = np.zeros((t_steps, 9, PR, PC), np.float32)
    xpad = np.zeros((t_steps, PR, PC), np.float32)
    xpad[:, :, 1:65] = x_local34
    for ky in range(3):
        for kx in range(3):
            k = ky * 3 + kx
            dr, dc = ky - 2, kx - 2
            r0, r1 = max(0, -dr), min(PR, PR - dr)
            c0, c1 = max(0, -dc), min(PC, PC - dc)
            xim[:, k, r0:r1, c0:c1] = xpad[:, r0 + dr:r1 + dr, c0 + dc:c1 + dc]
    return xim.reshape(t_steps, 9, PL)


def prep_core_inputs(x, w0, b0, w1, b1, core, t_steps):
    b, half = core // 2, core % 2
    xb = np.asarray(x[b, :t_steps, 0], np.float32)      # [T, 64, 64]
    x_local = np.zeros((t_steps, PR, W), np.float32)
    if half == 0:
        x_local[:, 1:34, :] = xb[:, 0:HH + 1, :]        # own rows + halo row 32
        w0f, w1f = w0, w1
    else:
        x_local[:, 1:34, :] = xb[:, HH - 1:2 * HH, :][:, ::-1, :]
        w0f, w1f = w0[:, :, ::-1, :], w1[:, :, ::-1, :]
    w0s, b0p = _pack_weights_l0(np.asarray(w0f, np.float32), np.asarray(b0, np.float32))
    w1s, b1p = _pack_weights_l1(np.asarray(w1f, np.float32), np.asarray(b1, np.float32))
    xim = _build_xim(x_local, t_steps)
    return {
        "w0s": np.ascontiguousarray(w0s),
        "w1s": np.ascontiguousarray(w1s),
        "b0p": np.ascontiguousarray(b0p),
        "b1p": np.ascontiguousarray(b1p),
        "xim": np.ascontiguousarray(xim),
        "zz": np.zeros((128, PL), np.float32),
    }


def assemble_output(results, t_steps):
    y = np.zeros((B, t_steps, HID, H, W), np.float32)
    for core in range(N_CORES):
        b, half = core // 2, core % 2
        ys = results[core]["y"].reshape(t_steps, HID, HH, W) * 0.5
        if half == 0:
            y[b, :, :, 0:HH, :] = ys
        else:
            y[b, :, :, HH:2 * HH, :] = ys[:, :, ::-1, :]
    return y


# ------------------------------------------------------------- device build

def _pview(t, p0, p1):
    """[p, PR, PC] view of a plane tile slice."""
    return t[p0:p1, :].rearrange("p (r c) -> p r c", r=PR, c=PC)


def _band(t, p0, p1, band):
    """[64, 16, 64] view of band `band` of a [128, 2048] tile slice."""
    return t[p0:p1, band * 1024:(band + 1) * 1024].rearrange(
        "p (r c) -> p r c", r=16, c=64)


def build_nc(t_steps):
    nc = bacc.Bacc("TRN2", target_bir_lowering=False, debug=False,
                   num_devices=N_CORES)
    d_w0s = nc.dram_tensor("w0s", [5, 2, 128, 128], F32R, kind="ExternalInput")
    d_w1s = nc.dram_tensor("w1s", [9, 2, 128, 128], F32R, kind="ExternalInput")
    d_b0 = nc.dram_tensor("b0p", [128, 2], F32, kind="ExternalInput")
    d_b1 = nc.dram_tensor("b1p", [128, 2], F32, kind="ExternalInput")
    d_xim = nc.dram_tensor("xim", [t_steps, 9, PL], F32R, kind="ExternalInput")
    d_zz = nc.dram_tensor("zz", [128, PL], F32R, kind="ExternalInput")
    d_y = nc.dram_tensor("y", [t_steps, HID, SP], F32R, kind="ExternalOutput")

    with tile.TileContext(nc) as tc:
        with (
            tc.tile_pool(name="persist", bufs=1) as pp,
            tc.tile_pool(name="gates", bufs=2) as gp,
            tc.tile_pool(name="psum", bufs=8, space="PSUM") as psp,
            tc.tile_pool(name="dram", bufs=2, space="DRAM") as dp,
        ):
            # persistent planes (f32r; matmul inputs)
            dbl1 = pp.tile([128, PL], F32R, tag="dbl1")
            dbl2 = pp.tile([128, PL], F32R, tag="dbl2")
            dbl3 = pp.tile([128, PL], F32R, tag="dbl3")
            l1in = pp.tile([128, PL], F32R, tag="l1in")
            # dbl1/dbl3 need no zero-init: t=0 uses only slot 4 (dbl2), and
            # the gates0(0) refresh fully rewrites them before conv0(1)
            for pl_t in (dbl2, l1in):
                nc.sync.dma_start(pl_t[:], d_zz.ap())
            # weights; slot 4 (the only t=0 slot) loads first
            w0t = pp.tile([128, 5 * 2 * 128], F32R, tag="w0t")
            nc.sync.dma_start(
                w0t[:, 8 * 128:10 * 128].rearrange(
                    "p (c m) -> p c m", c=2),
                d_w0s.ap()[4].rearrange("c k m -> k c m"))
            nc.sync.dma_start(
                w0t[:, 0:8 * 128].rearrange("p (s c m) -> p s c m", s=4, c=2),
                d_w0s.ap()[0:4].rearrange("s c k m -> k s c m"))
            w1t = pp.tile([128, 9 * 2 * 128], F32R, tag="w1t")
            nc.sync.dma_start(
                w1t[:].rearrange("p (s c m) -> p s c m", s=9, c=2),
                d_w1s.ap().rearrange("s c k m -> k s c m"))
            b0t = pp.tile([128, 2], F32, tag="b0t")
            nc.sync.dma_start(b0t[:], d_b0.ap())
            b1t = pp.tile([128, 2], F32, tag="b1t")
            nc.sync.dma_start(b1t[:], d_b1.ap())
            # first x im2col
            nc.sync.dma_start(dbl2[64:73, :], d_xim.ap()[0])

            def w0ap(sl, cb):
                return w0t[:, (sl * 2 + cb) * 128:(sl * 2 + cb + 1) * 128]

            def w1ap(si, cb):
                return w1t[:, (si * 2 + cb) * 128:(si * 2 + cb + 1) * 128]

            dbl_map = {"DBL1": dbl1, "DBL2": dbl2, "DBL3": dbl3}

            c_prev = gp.tile([128, SP], F32, tag="cpair")
            nc.any.memset(c_prev[:], 0.0)

            # --------------------------------------------------------------
            # Software-pipelined schedule. PE stream per iteration t:
            #   ... conv1(t) [bands 1,0], conv0(t+1) [bands 1,0] ...
            # gates1(t) (scalar/vector) overlaps conv0(t+1); gates0(t+1)
            # overlaps conv0(t+1)'s band-0 tail; the h0/h1 halo AllGathers
            # are kicked from a row-32-only mini gate chain right after the
            # producing band's psums land, hiding CC latency under matmuls.
            # PSUM tiles are [128,512] (1 bank) so conv1(t)/conv0(t+1)
            # lifetimes interleave within the 8 banks.
            # --------------------------------------------------------------

            def conv0(t):
                """Layer-0 conv for step t -> {(band, cb, sl): psum}."""
                ps0 = {}
                for band in (1, 0):
                    for cb in range(2):
                        for sl in range(2):
                            ps = psp.tile([128, 512], F32)
                            ps0[(band, cb, sl)] = ps
                            r0 = band * 16 + sl * 8 + 1
                            slots = range(5) if t > 0 else [4]
                            first = 0 if t > 0 else 4
                            for slot in slots:
                                sa, sb_, tname = SLOTS0[slot]
                                src = dbl_map[tname]
                                rhs = _pview(src, 0, 128)[
                                    :, r0 + sa[0]:r0 + sa[0] + 8,
                                    1 + sa[1]:1 + sa[1] + 64]
                                nc.tensor.matmul(
                                    ps[:], w0ap(slot, cb), rhs,
                                    start=(slot == first), stop=(slot == 4))
                return ps0

            def conv1(t):
                """Layer-1 conv for step t -> {(band, cb, sl): psum}.

                Shift order defers cross-band / halo row reads: band-1 sl0
                does dy=-1 (reads row 16 <- gates0 band 0) last; band-1 sl1
                does dy=+1 (reads halo row 33 <- AllGather) last.
                """
                ps1 = {}
                for band in (1, 0):
                    for cb in range(2):
                        for sl in range(2):
                            ps = psp.tile([128, 512], F32)
                            ps1[(band, cb, sl)] = ps
                            r0 = band * 16 + sl * 8 + 1
                            if band == 1 and sl == 0:
                                order = [s for s in SHIFTS if s[0] != -1] + \
                                        [s for s in SHIFTS if s[0] == -1]
                            elif band == 1 and sl == 1:
                                order = [s for s in SHIFTS if s[0] != 1] + \
                                        [s for s in SHIFTS if s[0] == 1]
                            else:
                                order = SHIFTS
                            for si, (dy, dx) in enumerate(order):
                                rhs = _pview(l1in, 0, 128)[
                                    :, r0 + dy:r0 + dy + 8, 1 + dx:1 + dx + 64]
                                nc.tensor.matmul(
                                    ps[:], w1ap(SHIFTS.index((dy, dx)), cb),
                                    rhs, start=(si == 0), stop=(si == 8))
                return ps1

            def halo_send(half, tag):
                """AllGather plane row 32 of l1in half -> returns recv state."""
                p0, p1 = (64, 128) if half else (0, 64)
                ag_in = dp.tile([64, 64], F32R, tag=f"ag{tag}i")
                ag_out = dp.tile([128, 64], F32R, tag=f"ag{tag}o")
                nc.sync.dma_start(ag_in[:], _pview(l1in, p0, p1)[:, 32, 1:65])
                nc.gpsimd.collective_compute(
                    "AllGather", AluOpType.bypass,
                    replica_groups=REPLICA_GROUPS,
                    ins=[ag_in.opt()], outs=[ag_out.opt()])
                return ag_out

            def halo_recv(half, tag, ag_out):
                """Write peer's row into l1in plane row 33 (sum-minus-own)."""
                p0, p1 = (64, 128) if half else (0, 64)
                agt = gp.tile([128, 128], F32R, tag=f"agt{tag}")
                nc.sync.dma_start(
                    agt[p0:p1, :].rearrange("p (s c) -> p s c", s=2),
                    ag_out[:].rearrange("(s p) c -> p s c", s=2))
                st = gp.tile([128, 64], F32R, tag=f"st{tag}")
                nc.vector.tensor_add(st[p0:p1, :], agt[p0:p1, 0:64],
                                     agt[p0:p1, 64:128])
                nc.vector.tensor_sub(
                    _pview(l1in, p0, p1)[:, 33, 1:65], st[p0:p1, :],
                    _pview(l1in, p0, p1)[:, 32, 1:65])

            def chain0a(ps0, c_new, sig0, tgo0, t2p, band, sl, c0, c1):
                """Layer-0 gates phase A: activations, c-muls, accum-DMA.

                SP cols = band*1024+sl*512+c0. The i*tanh(g) mul runs on
                gpsimd right before its accum on the same queue."""
                s0 = band * 1024 + sl * 512 + c0
                bs = slice(s0, s0 + (c1 - c0))
                nc.scalar.activation(sig0[:, bs], ps0[(band, 0, sl)][:, c0:c1],
                                     ACT_SIG, bias=b0t[:, 0:1])
                nc.scalar.activation(tgo0[:, bs], ps0[(band, 1, sl)][:, c0:c1],
                                     ACT_TANH, bias=b0t[:, 1:2])
                nc.vector.tensor_mul(c_new[64:128, bs], sig0[64:128, bs],
                                     c_prev_ref[0][64:128, bs])
                nc.vector.tensor_mul(t2p[0:64, bs], sig0[0:64, bs],
                                     tgo0[0:64, bs])
                nc.gpsimd.dma_start(c_new[64:128, bs], t2p[0:64, bs],
                                    accum_op=AluOpType.add)

            def chain0b(c_new, tgo0, thc, band, sl, c0, c1):
                """Layer-0 gates phase B: tanh(c0) and h0' -> l1in hi."""
                s0 = band * 1024 + sl * 512 + c0
                bs = slice(s0, s0 + (c1 - c0))
                nc.scalar.activation(thc[64:128, bs], c_new[64:128, bs],
                                     ACT_TANH)
                r0 = band * 16 + sl * 8 + 1 + c0 // 64
                rows = _pview(l1in, 64, 128)[:, r0:r0 + (c1 - c0) // 64, 1:65]
                nc.vector.scalar_tensor_tensor(
                    rows,
                    tgo0[64:128, bs].rearrange("p (r c) -> p r c", c=64), 1.0,
                    thc[64:128, bs].rearrange("p (r c) -> p r c", c=64),
                    AluOpType.add, AluOpType.mult)

            def chain1a(ps1, c_new, sig1, og1, t2p, band, sl, c0, c1):
                """Layer-1 gates phase A."""
                s0 = band * 1024 + sl * 512 + c0
                bs = slice(s0, s0 + (c1 - c0))
                nc.scalar.activation(sig1[:, bs], ps1[(band, 0, sl)][:, c0:c1],
                                     ACT_SIG, bias=b1t[:, 0:1])
                nc.scalar.activation(og1[:, bs], ps1[(band, 1, sl)][:, c0:c1],
                                     ACT_TANH, bias=b1t[:, 1:2])
                nc.vector.tensor_mul(c_new[0:64, bs], sig1[0:64, bs],
                                     c_prev_ref[0][0:64, bs])
                nc.vector.tensor_mul(t2p[64:128, bs], sig1[64:128, bs],
                                     og1[64:128, bs])
                nc.gpsimd.dma_start(c_new[0:64, bs], t2p[64:128, bs],
                                    accum_op=AluOpType.add)

            def chain1b(c_new, og1, thc, band, sl, c0, c1):
                """Layer-1 gates phase B: tanh(c1), h1' = 2*h1 -> l1in lo."""
                s0 = band * 1024 + sl * 512 + c0
                bs = slice(s0, s0 + (c1 - c0))
                nc.scalar.activation(thc[0:64, bs], c_new[0:64, bs],
                                     ACT_TANH)
                r0 = band * 16 + sl * 8 + 1 + c0 // 64
                rows = _pview(l1in, 0, 64)[:, r0:r0 + (c1 - c0) // 64, 1:65]
                # h1' = 2*h1 = (tanh(o/2)+1)*tanh(c1); host halves y
                nc.vector.scalar_tensor_tensor(
                    rows,
                    og1[0:64, bs].rearrange("p (r c) -> p r c", c=64), 1.0,
                    thc[0:64, bs].rearrange("p (r c) -> p r c", c=64),
                    AluOpType.add, AluOpType.mult)

            c_prev_ref = [c_prev]

            # prologue: layer-0 conv for t=0 (x taps only; h is zero)
            ps0 = conv0(0)

            for t in range(t_steps):
                # ---- gates0(t): band 1 (mini row 32 first -> AG), band 0
                sig0 = gp.tile([128, SP], F32, tag="sig0")
                tgo0 = gp.tile([128, SP], F32, tag="tgo0")
                c_new = gp.tile([128, SP], F32, tag="cpair")
                t2p = gp.tile([128, SP], F32, tag="t2p")
                thc = gp.tile([128, SP], F32, tag="thc")

                # mini row-32 chain first -> early AllGather kick
                chain0a(ps0, c_new, sig0, tgo0, t2p, 1, 1, 448, 512)
                chain0b(c_new, tgo0, thc, 1, 1, 448, 512)
                ag0 = halo_send(1, "0")
                # A/B staggered so the scalar queue never waits on accum-DMAs;
                # (1,1)/(1,0) complete first (conv1 band 1 needs rows 17-32)
                chain0a(ps0, c_new, sig0, tgo0, t2p, 1, 1, 0, 448)
                chain0a(ps0, c_new, sig0, tgo0, t2p, 1, 0, 0, 512)
                chain0b(c_new, tgo0, thc, 1, 1, 0, 448)
                chain0b(c_new, tgo0, thc, 1, 0, 0, 512)
                # recv here: the AG is done by now, and agt0 must precede
                # the refresh copies on the sync queue so conv1's deferred
                # halo matmuls aren't pushed past conv0(t+1)
                halo_recv(1, "0", ag0)
                # band 0: sl1 first (conv1 band 1 needs plane row 16)
                chain0a(ps0, c_new, sig0, tgo0, t2p, 0, 1, 0, 512)
                chain0a(ps0, c_new, sig0, tgo0, t2p, 0, 0, 0, 512)
                chain0b(c_new, tgo0, thc, 0, 1, 0, 512)
                chain0b(c_new, tgo0, thc, 0, 0, 0, 512)

                # refresh layer-0 rhs planes for step t+1 (feeds conv0(t+1)).
                # Big copies stop before plane row 33 (flat col 33*66=2178)
                # so they don't wait on the halo; tiny row-33 tails go after
                # halo_recv below.
                R33 = 33 * 66
                if t + 1 < t_steps:
                    nc.sync.dma_start(dbl1[0:64, 0:R33], l1in[64:128, 0:R33])
                    nc.sync.dma_start(dbl1[64:128, 0:R33],
                                      l1in[64:128, 1:R33 + 1])
                    nc.sync.dma_start(dbl3[0:64, 0:R33], l1in[64:128, 0:R33])
                    nc.sync.dma_start(dbl3[64:128, 0:R33 - 66],
                                      l1in[64:128, 66:R33])
                    nc.sync.dma_start(dbl2[0:64, 0:R33], l1in[64:128, 0:R33])
                    nc.sync.dma_start(dbl2[64:73, :], d_xim.ap()[t + 1])

                # row-33 tails of the dbl refresh (need the halo row)
                if t + 1 < t_steps:
                    nc.sync.dma_start(dbl1[0:64, R33:PL], l1in[64:128, R33:PL])
                    nc.sync.dma_start(dbl1[64:128, R33:PL - 1],
                                      l1in[64:128, R33 + 1:PL])
                    nc.sync.dma_start(dbl3[0:64, R33:PL], l1in[64:128, R33:PL])
                    nc.sync.dma_start(dbl3[64:128, R33 - 66:R33],
                                      l1in[64:128, R33:PL])
                    nc.sync.dma_start(dbl2[0:64, R33:PL], l1in[64:128, R33:PL])

                # ---- conv1(t) on PE (after conv0(t) in the queue)
                ps1 = conv1(t)

                # ---- gates1(t): overlaps conv0(t+1) on PE
                sig1 = gp.tile([128, SP], F32, tag="sig1")
                og1 = gp.tile([128, SP], F32, tag="og1")

                chain1a(ps1, c_new, sig1, og1, t2p, 1, 1, 448, 512)
                chain1b(c_new, og1, thc, 1, 1, 448, 512)
                if t + 1 < t_steps:
                    ag1 = halo_send(0, "1")
                chain1a(ps1, c_new, sig1, og1, t2p, 1, 1, 0, 448)
                chain1a(ps1, c_new, sig1, og1, t2p, 1, 0, 0, 512)
                chain1b(c_new, og1, thc, 1, 1, 0, 448)
                chain1b(c_new, og1, thc, 1, 0, 0, 512)
                # y band 1
                nc.sync.dma_start(
                    d_y.ap()[t][:, 1024:2048].rearrange(
                        "p (r c) -> p r c", r=16, c=64),
                    _pview(l1in, 0, 64)[:, 17:33, 1:65])
                chain1a(ps1, c_new, sig1, og1, t2p, 0, 1, 0, 512)
                chain1a(ps1, c_new, sig1, og1, t2p, 0, 0, 0, 512)
                chain1b(c_new, og1, thc, 0, 1, 0, 512)
                chain1b(c_new, og1, thc, 0, 0, 0, 512)
                if t + 1 < t_steps:
                    halo_recv(0, "1", ag1)
                # y band 0
                nc.sync.dma_start(
                    d_y.ap()[t][:, 0:1024].rearrange(
                        "p (r c) -> p r c", r=16, c=64),
                    _pview(l1in, 0, 64)[:, 1:17, 1:65])

                # ---- conv0(t+1) on PE (independent of gates1(t))
                if t + 1 < t_steps:
                    ps0 = conv0(t + 1)

                c_prev_ref[0] = c_new

    nc.compile()
    return nc


# ------------------------------------------------------------------ driver

def _ensure_axon_ntff_hook():
    """Install the NTFF profile hook bass_utils expects under axon, if the
    environment's antenv lacks it. Only used when tracing is requested."""
    import sys as _sys
    import types as _types
    import ctypes as _ctypes
    import contextlib as _contextlib

    try:
        from antenv.axon_hooks import get_axon_ntff_profile_hook  # noqa: F401
        return
    except ImportError:
        pass
    so_path = "/opt/axon/libaxon_pjrt.so"
    if not os.path.exists(so_path):
        return
    lib = _ctypes.CDLL(so_path)
    if not hasattr(lib, "axon_start_nrt_profile"):
        return
    lib.axon_start_nrt_profile.argtypes = [
        _ctypes.POINTER(_ctypes.c_int64), _ctypes.c_size_t]
    lib.axon_start_nrt_profile.restype = _ctypes.c_int64
    lib.axon_stop_nrt_profile.argtypes = [_ctypes.c_char_p]
    lib.axon_stop_nrt_profile.restype = _ctypes.c_int64

    @_contextlib.contextmanager
    def _hook(output_dir, device_ids):
        import jax
        jax.devices()
        if device_ids:
            ids = (_ctypes.c_int64 * len(device_ids))(*device_ids)
            rc = lib.axon_start_nrt_profile(ids, len(device_ids))
        else:
            rc = lib.axon_start_nrt_profile(None, 0)
        if rc != 0:
            raise RuntimeError(f"axon_start_nrt_profile rc={rc}")
        try:
            yield
        finally:
            n = lib.axon_stop_nrt_profile(str(output_dir).encode())
            print(f"ntff profile: {n} file(s) -> {output_dir}")

    mod = _types.ModuleType("antenv.axon_hooks")
    mod.get_axon_ntff_profile_hook = lambda: _hook
    import antenv  # noqa: F401
    _sys.modules["antenv.axon_hooks"] = mod


_CACHE = {}


def _get_nc(t_steps):
    if t_steps not in _CACHE:
        _CACHE[t_steps] = build_nc(t_steps)
    return _CACHE[t_steps]


def run_cores(x, w0, b0, w1, b1, t_steps=None, trace=False, tmpdir=None):
    t_steps = t_steps or x.shape[1]
    nc = _get_nc(t_steps)
    in_maps = [prep_core_inputs(x, w0, b0, w1, b1, core, t_steps)
               for core in range(N_CORES)]
    kwargs = {}
    if trace:
        _ensure_axon_ntff_hook()
        bass_utils.upload_artifacts = lambda d: d  # no artifact bucket here
        if tmpdir:
            kwargs["tmpdir"] = tmpdir
    res = bass_utils.run_bass_kernel_spmd(
        nc, in_maps, core_ids=list(range(N_CORES)), trace=trace, **kwargs)
    return res


def kernel(x, w0, b0, w1, b1):
    x = np.asarray(x, np.float32)
    t_steps = x.shape[1]
    trace = bool(int(os.environ.get("CONVLSTM_TRACE", "0")))
    res = run_cores(x, np.asarray(w0, np.float32), np.asarray(b0, np.float32),
                    np.asarray(w1, np.float32), np.asarray(b1, np.float32),
                    t_steps=t_steps, trace=trace)
    kernel.last_results = res
    return assemble_output(res.results, t_steps)

